# revision 6
# baseline (speedup 1.0000x reference)
"""Trainium2 Bass kernel for nn_ContinuousMamba (v2).

Sharding: 8 cores = 4 batches x 2 halves of d_inner (1536 -> 768/core).
Core c handles batch c//2, channel half c%2; the pair AllReduces the
x_proj partial and the out_proj partial per time-slice.

v2 design (vs v1 baseline):
- bf16 everywhere off the PE/psum path (fp32 matmul runs 2-pass; bf16 1-pass,
  and DVE gets 2x/4x modes on packed bf16).
- Time sliced into 3 strips of ~342 used uniformly for matmuls, the scan
  chunks, and the (bf16) AllReduces, so layers pipeline in a wavefront.
- dA = exp(A*delta) computed directly by per-(m,n) scaled EXP on the scalar
  engine (no outer-product materialization); delta via native Softplus.
- dBu broadcast-mult and the C*H mult run on GpSimd (otherwise idle);
  the n-reduction is a bf16 add-tree on vector (cheaper than TENSOR_REDUCE).
- u/z/delta/y stay resident in SBUF (no DRAM round trips).
"""

import sys

sys.path.insert(0, "/opt/trn_rl_repo")

import numpy as np

import concourse.bass as bass
import concourse.tile as tile
from concourse import bacc, mybir
from concourse.bass import AP

FP32 = mybir.dt.float32
BF16 = mybir.dt.bfloat16
AF = mybir.ActivationFunctionType
OP = mybir.AluOpType


def full_cfg():
    return dict(
        n_cores=8,
        d_model=768,
        d_half=768,
        d_state=16,
        dt_rank=48,
        n_layers=4,
        L=1025,  # 1 x-token + 1024 v-tokens
        d_conv=4,
        strips=[342, 342, 341],
        dbu_on_gp=True,
        cmul_on_gp=True,
    )


def _ap(base: AP, extra_offset: int, dims):
    return AP(tensor=base.tensor, offset=base.offset + extra_offset, ap=list(dims))


def build(cfg):
    dm = cfg["d_model"]
    dh = cfg["d_half"]
    ns = cfg["d_state"]
    dtr = cfg["dt_rank"]
    nl = cfg["n_layers"]
    L = cfg["L"]
    dconv = cfg["d_conv"]
    strips = cfg["strips"]
    assert sum(strips) == L
    km = dm // 128
    mh = dh // 128
    tcmax = max(strips)
    nx = dtr + 2 * ns  # 80
    nstrip = len(strips)
    spans = []
    t0 = 0
    for ts_ in strips:
        spans.append((t0, ts_))
        t0 += ts_

    nc = bacc.Bacc("TRN2", target_bir_lowering=False, debug=False,
                   num_devices=cfg["n_cores"])

    # ---- I/O ----
    xv6 = nc.dram_tensor("xv6", [6, L], BF16, kind="ExternalInput")
    w6 = nc.dram_tensor("w6", [6, dm], BF16, kind="ExternalInput")
    w_in = nc.dram_tensor("w_in", [nl, dm, 2 * dh], BF16, kind="ExternalInput")
    w_cv = nc.dram_tensor("w_cv", [nl, dh, dconv], FP32, kind="ExternalInput")
    b_cv = nc.dram_tensor("b_cv", [nl, dh], FP32, kind="ExternalInput")
    w_xp = nc.dram_tensor("w_xp", [nl, dh, nx], BF16, kind="ExternalInput")
    w_dt = nc.dram_tensor("w_dt", [nl, dtr, dh], BF16, kind="ExternalInput")
    b_dt = nc.dram_tensor("b_dt", [nl, dh], FP32, kind="ExternalInput")
    alog = nc.dram_tensor("alog", [nl, dh, ns], FP32, kind="ExternalInput")
    dskp = nc.dram_tensor("dskp", [nl, dh], FP32, kind="ExternalInput")
    w_ou = nc.dram_tensor("w_ou", [nl, dh, dm], BF16, kind="ExternalInput")
    w_hd = nc.dram_tensor("w_hd", [dm, 2], BF16, kind="ExternalInput")
    b_hd = nc.dram_tensor("b_hd", [2], FP32, kind="ExternalInput")
    yout = nc.dram_tensor("yout", [2, L - 1], FP32, kind="ExternalOutput")

    with tile.TileContext(nc) as tc:
        import contextlib

        ctx = contextlib.ExitStack()
        with ctx:
            strip_p = ctx.enter_context(tc.tile_pool(name="strip", bufs=2))
            h_p = ctx.enter_context(tc.tile_pool(name="hp", bufs=4))
            scan_p = ctx.enter_context(tc.tile_pool(name="scan", bufs=2))
            bc_p = ctx.enter_context(tc.tile_pool(name="bc", bufs=1))
            w_p = ctx.enter_context(tc.tile_pool(name="w", bufs=2))
            lw_p = ctx.enter_context(tc.tile_pool(name="lw", bufs=2))
            sm_p = ctx.enter_context(tc.tile_pool(name="sm", bufs=2))
            carry_p = ctx.enter_context(tc.tile_pool(name="carry", bufs=2))
            ps = ctx.enter_context(tc.tile_pool(name="ps", bufs=6, space="PSUM"))
            dram = ctx.enter_context(tc.tile_pool(name="dram", bufs=3, space="DRAM"))

            groups = [[2 * i, 2 * i + 1] for i in range(cfg["n_cores"] // 2)]

            # ---------- embeddings -> h strips (layer 0 input) ----------
            xv_sb = lw_p.tile([6, L], BF16, tag="xv")
            nc.sync.dma_start(out=xv_sb[:], in_=xv6[:])
            w6_sb = lw_p.tile([6, dm], BF16, tag="w6")
            nc.sync.dma_start(out=w6_sb[:], in_=w6[:])

            h_tiles = [None] * nstrip
            for si, (t0, ts_) in enumerate(spans):
                h_s = h_p.tile([128, km, tcmax], BF16, tag="h", name=f"h0_{si}")
                for k in range(km):
                    pt = ps.tile([128, 512], FP32, tag="ps")
                    nc.tensor.matmul(out=pt[:, :ts_],
                                     lhsT=w6_sb[:, k * 128:(k + 1) * 128],
                                     rhs=xv_sb[:, t0:t0 + ts_])
                    nc.scalar.copy(out=h_s[:, k, :ts_], in_=pt[:, :ts_])
                h_tiles[si] = h_s

            # ---------- layers ----------
            hn_slices = [None] * nstrip
            for l in range(nl):
                with nc.named_scope(f"layer{l}"):
                    # --- per-layer small params ---
                    wcv_sb = lw_p.tile([128, mh, dconv], FP32, tag="wcv")
                    nc.sync.dma_start(
                        out=wcv_sb[:],
                        in_=_ap(w_cv.ap(), l * dh * dconv,
                                [[dconv, 128], [128 * dconv, mh], [1, dconv]]))
                    bcv_sb = lw_p.tile([128, mh], FP32, tag="bcv")
                    nc.sync.dma_start(out=bcv_sb[:],
                                      in_=_ap(b_cv.ap(), l * dh, [[1, 128], [128, mh]]))
                    bdt_sb = lw_p.tile([128, mh], FP32, tag="bdt")
                    nc.sync.dma_start(out=bdt_sb[:],
                                      in_=_ap(b_dt.ap(), l * dh, [[1, 128], [128, mh]]))
                    dsk_sb = lw_p.tile([128, mh], FP32, tag="dsk")
                    nc.sync.dma_start(out=dsk_sb[:],
                                      in_=_ap(dskp.ap(), l * dh, [[1, 128], [128, mh]]))
                    a_sb = lw_p.tile([128, mh, ns], FP32, tag="a")
                    nc.sync.dma_start(
                        out=a_sb[:],
                        in_=_ap(alog.ap(), l * dh * ns,
                                [[ns, 128], [128 * ns, mh], [1, ns]]))
                    a_flat = a_sb.rearrange("p m n -> p (m n)")
                    nc.scalar.activation(out=a_flat, in_=a_flat, func=AF.Exp)
                    nc.scalar.mul(out=a_flat, in_=a_flat, mul=-1.0)
                    wxp_sb = lw_p.tile([128, mh, nx], BF16, tag="wxp")
                    nc.sync.dma_start(
                        out=wxp_sb[:],
                        in_=_ap(w_xp.ap(), l * dh * nx,
                                [[nx, 128], [128 * nx, mh], [1, nx]]))
                    wdt_sb = lw_p.tile([dtr, dh], BF16, tag="wdt")
                    nc.sync.dma_start(out=wdt_sb[:],
                                      in_=_ap(w_dt.ap(), l * dtr * dh,
                                              [[dh, dtr], [1, dh]]))

                    carries = [None] * mh
                    xz_prev = None
                    for si, (t0, ts_) in enumerate(spans):
                        # --- load h strip (layers > 0) ---
                        if l == 0:
                            h_s = h_tiles[si]
                        else:
                            h_s = h_p.tile([128, km, tcmax], BF16, tag="h",
                                           name=f"h{l}_{si}")
                            for k in range(km):
                                nc.sync.dma_start(
                                    out=h_s[:, k, :ts_],
                                    in_=hn_slices[si][k * 128:(k + 1) * 128, :ts_])

                        # --- in_proj (x into xz_s with 3-col halo, z silu'd) ---
                        xz_s = strip_p.tile([128, mh, dconv - 1 + tcmax], BF16,
                                            tag="xz", name=f"xz{l}_{si}")
                        z_s = strip_p.tile([128, mh, tcmax], BF16, tag="z",
                                           name=f"z{l}_{si}")
                        for m in range(mh):
                            winx = w_p.tile([128, km, 128], BF16, tag="winx")
                            nc.sync.dma_start(
                                out=winx[:],
                                in_=_ap(w_in.ap(), l * dm * 2 * dh + m * 128,
                                        [[2 * dh, 128], [128 * 2 * dh, km], [1, 128]]))
                            winz = w_p.tile([128, km, 128], BF16, tag="winz")
                            nc.sync.dma_start(
                                out=winz[:],
                                in_=_ap(w_in.ap(), l * dm * 2 * dh + dh + m * 128,
                                        [[2 * dh, 128], [128 * 2 * dh, km], [1, 128]]))
                            ptx = ps.tile([128, 512], FP32, tag="ps")
                            ptz = ps.tile([128, 512], FP32, tag="ps")
                            for k in range(km):
                                nc.tensor.matmul(out=ptx[:, :ts_], lhsT=winx[:, k, :],
                                                 rhs=h_s[:, k, :ts_],
                                                 start=(k == 0), stop=(k == km - 1))
                            for k in range(km):
                                nc.tensor.matmul(out=ptz[:, :ts_], lhsT=winz[:, k, :],
                                                 rhs=h_s[:, k, :ts_],
                                                 start=(k == 0), stop=(k == km - 1))
                            # halo: 3 cols from previous strip (zeros at t=0)
                            if si == 0:
                                nc.vector.memset(xz_s[:, m, 0:dconv - 1], 0.0)
                            else:
                                nc.vector.tensor_copy(
                                    out=xz_s[:, m, 0:dconv - 1],
                                    in_=xz_prev[:, m,
                                                strips[si - 1]:strips[si - 1] + dconv - 1])
                            nc.scalar.copy(out=xz_s[:, m, dconv - 1:dconv - 1 + ts_],
                                           in_=ptx[:, :ts_])
                            nc.scalar.activation(out=z_s[:, m, :ts_], in_=ptz[:, :ts_],
                                                 func=AF.Silu)
                        xz_prev = xz_s

                        # --- conv + silu -> u ---
                        u_s = strip_p.tile([128, mh, tcmax], BF16, tag="u",
                                           name=f"u{l}_{si}")
                        for m in range(mh):
                            cv = sm_p.tile([128, tcmax], BF16, tag="cv")
                            nc.vector.tensor_scalar(
                                out=cv[:, :ts_], in0=xz_s[:, m, 0:ts_],
                                scalar1=wcv_sb[:, m, 0:1], scalar2=bcv_sb[:, m:m + 1],
                                op0=OP.mult, op1=OP.add)
                            for k in range(1, dconv):
                                nc.vector.scalar_tensor_tensor(
                                    out=cv[:, :ts_], in0=xz_s[:, m, k:k + ts_],
                                    scalar=wcv_sb[:, m, k:k + 1], in1=cv[:, :ts_],
                                    op0=OP.mult, op1=OP.add)
                            nc.scalar.activation(out=u_s[:, m, :ts_], in_=cv[:, :ts_],
                                                 func=AF.Silu)

                        # --- x_proj partial -> DRAM -> AllReduce ---
                        ptp = ps.tile([128, 512], FP32, tag="ps")
                        for k in range(mh):
                            nc.tensor.matmul(out=ptp[0:nx, :ts_], lhsT=wxp_sb[:, k, :],
                                             rhs=u_s[:, k, :ts_],
                                             start=(k == 0), stop=(k == mh - 1))
                        xpp_sb = sm_p.tile([nx, tcmax], BF16, tag="xpp")
                        nc.scalar.copy(out=xpp_sb[:, :ts_], in_=ptp[0:nx, :ts_])
                        xpp_d = dram.tile([nx, ts_], BF16, tag="xpp")
                        nc.sync.dma_start(out=xpp_d[:], in_=xpp_sb[:, :ts_])
                        xpr_d = dram.tile([nx, ts_], BF16, tag="xpr")
                        nc.gpsimd.collective_compute(
                            "AllReduce", OP.add, replica_groups=groups,
                            ins=[xpp_d.opt()], outs=[xpr_d.opt()])

                        # --- dt rows -> delta (softplus); B/C broadcast ---
                        xdt_sb = sm_p.tile([dtr, tcmax], BF16, tag="xdt")
                        nc.sync.dma_start(out=xdt_sb[:, :ts_], in_=xpr_d[0:dtr, :])
                        b_ball = bc_p.tile([128, ns * tcmax], BF16, tag="bball")
                        c_ball = bc_p.tile([128, ns * tcmax], BF16, tag="cball")
                        nc.sync.dma_start(
                            out=b_ball[:, :ns * ts_],
                            in_=_ap(xpr_d[:], dtr * ts_,
                                    [[0, 128], [ts_, ns], [1, ts_]]))
                        nc.sync.dma_start(
                            out=c_ball[:, :ns * ts_],
                            in_=_ap(xpr_d[:], (dtr + ns) * ts_,
                                    [[0, 128], [ts_, ns], [1, ts_]]))
                        dl_s = strip_p.tile([128, mh, tcmax], BF16, tag="dl",
                                            name=f"dl{l}_{si}")
                        for m in range(mh):
                            ptd = ps.tile([128, 512], FP32, tag="ps")
                            nc.tensor.matmul(out=ptd[:, :ts_],
                                             lhsT=wdt_sb[:, m * 128:(m + 1) * 128],
                                             rhs=xdt_sb[:, :ts_])
                            spe = sm_p.tile([128, tcmax], BF16, tag="spe")
                            nc.scalar.activation(out=spe[:, :ts_],
                                                 in_=ptd[:, :ts_], func=AF.Exp,
                                                 bias=bdt_sb[:, m:m + 1])
                            nc.scalar.activation(out=dl_s[:, m, :ts_],
                                                 in_=spe[:, :ts_], func=AF.Ln,
                                                 bias=1.0)
                        du_s = strip_p.tile([128, mh, tcmax], BF16, tag="du",
                                            name=f"du{l}_{si}")
                        nc.vector.tensor_mul(
                            out=du_s.rearrange("p m t -> p (m t)"),
                            in0=dl_s.rearrange("p m t -> p (m t)"),
                            in1=u_s.rearrange("p m t -> p (m t)"))

                        # --- scan per m-chunk ---
                        y_s = strip_p.tile([128, mh, tcmax], BF16, tag="y",
                                           name=f"y{l}_{si}")
                        for m in range(mh):
                            da = scan_p.tile([128, ns * tcmax], BF16, tag="da")
                            for n in range(ns):
                                nc.scalar.activation(
                                    out=da[:, n * ts_:(n + 1) * ts_],
                                    in_=dl_s[:, m, :ts_], func=AF.Exp,
                                    scale=a_sb[:, m, n:n + 1])
                            dbu = scan_p.tile([128, ns * tcmax], BF16, tag="dbu")
                            du_bc = _ap(du_s[:], m * tcmax,
                                        [du_s[:].ap[0], [0, ns], [1, ts_]])
                            eng = nc.gpsimd if cfg["dbu_on_gp"] else nc.vector
                            eng.tensor_tensor(out=dbu[:, :ns * ts_], in0=du_bc,
                                              in1=b_ball[:, :ns * ts_], op=OP.mult)
                            if si > 0:
                                da0 = sm_p.tile([128, ns], BF16, tag="da0")
                                nc.vector.tensor_copy(
                                    out=da0[:],
                                    in_=_ap(da[:], 0, [da[:].ap[0], [ts_, ns]]))
                                inj = sm_p.tile([128, ns], BF16, tag="inj")
                                nc.vector.tensor_mul(out=inj[:], in0=da0[:],
                                                     in1=carries[m][:])
                                dbu0 = _ap(dbu[:], 0, [dbu[:].ap[0], [ts_, ns]])
                                nc.vector.tensor_tensor(out=dbu0, in0=dbu0,
                                                        in1=inj[:], op=OP.add)
                            nc.vector.memset(
                                _ap(da[:], 0, [da[:].ap[0], [ts_, ns]]), 0.0)
                            hsc = scan_p.tile([128, ns * tcmax], BF16, tag="hsc")
                            nc.vector.tensor_tensor_scan(
                                out=hsc[:, :ns * ts_], data0=da[:, :ns * ts_],
                                data1=dbu[:, :ns * ts_], initial=0.0,
                                op0=OP.mult, op1=OP.add)
                            if si < nstrip - 1:
                                ncar = carry_p.tile([128, ns], BF16, tag=f"car{m}",
                                                    name=f"car{l}_{si}_{m}")
                                nc.vector.tensor_copy(
                                    out=ncar[:],
                                    in_=_ap(hsc[:], ts_ - 1, [hsc[:].ap[0], [ts_, ns]]))
                                carries[m] = ncar
                            # ch = C * H (into dbu; hsc dead after tree lvl reads)
                            ceng = nc.gpsimd if cfg["cmul_on_gp"] else nc.vector
                            ceng.tensor_tensor(out=dbu[:, :ns * ts_],
                                               in0=hsc[:, :ns * ts_],
                                               in1=c_ball[:, :ns * ts_], op=OP.mult)
                            # add-tree over n: dbu -> hsc -> dbu -> hsc -> y
                            nc.vector.tensor_add(out=hsc[:, 0:8 * ts_],
                                                 in0=dbu[:, 0:8 * ts_],
                                                 in1=dbu[:, 8 * ts_:16 * ts_])
                            nc.vector.tensor_add(out=dbu[:, 0:4 * ts_],
                                                 in0=hsc[:, 0:4 * ts_],
                                                 in1=hsc[:, 4 * ts_:8 * ts_])
                            nc.vector.tensor_add(out=hsc[:, 0:2 * ts_],
                                                 in0=dbu[:, 0:2 * ts_],
                                                 in1=dbu[:, 2 * ts_:4 * ts_])
                            nc.vector.tensor_add(out=y_s[:, m, :ts_],
                                                 in0=hsc[:, 0:ts_],
                                                 in1=hsc[:, ts_:2 * ts_])
                            # y += u * D_skip
                            nc.vector.scalar_tensor_tensor(
                                out=y_s[:, m, :ts_], in0=u_s[:, m, :ts_],
                                scalar=dsk_sb[:, m:m + 1], in1=y_s[:, m, :ts_],
                                op0=OP.mult, op1=OP.add)
                        # y *= silu(z)
                        nc.gpsimd.tensor_tensor(
                            out=y_s.rearrange("p m t -> p (m t)"),
                            in0=y_s.rearrange("p m t -> p (m t)"),
                            in1=z_s.rearrange("p m t -> p (m t)"), op=OP.mult)

                        # --- out_proj partial for this strip -> AllReduce ---
                        hp_d = dram.tile([dm, ts_], BF16, tag="hpp")
                        for mo in range(km):
                            wou_t = w_p.tile([128, mh, 128], BF16, tag="wou")
                            nc.sync.dma_start(
                                out=wou_t[:],
                                in_=_ap(w_ou.ap(), l * dh * dm + mo * 128,
                                        [[dm, 128], [128 * dm, mh], [1, 128]]))
                            po = ps.tile([128, 512], FP32, tag="ps")
                            for k in range(mh):
                                nc.tensor.matmul(out=po[:, :ts_],
                                                 lhsT=wou_t[:, k, :],
                                                 rhs=y_s[:, k, :ts_],
                                                 start=(k == 0), stop=(k == mh - 1))
                            ho = sm_p.tile([128, tcmax], BF16, tag="ho")
                            nc.scalar.copy(out=ho[:, :ts_], in_=po[:, :ts_])
                            nc.sync.dma_start(out=hp_d[mo * 128:(mo + 1) * 128, :],
                                              in_=ho[:, :ts_])
                        hn_d = dram.tile([dm, ts_], BF16, tag="hnr")
                        nc.gpsimd.collective_compute(
                            "AllReduce", OP.add, replica_groups=groups,
                            ins=[hp_d.opt()], outs=[hn_d.opt()])
                        hn_slices[si] = hn_d

            # ---------- head ----------
            whd_sb = lw_p.tile([128, km, 2], BF16, tag="whd")
            nc.sync.dma_start(out=whd_sb[:],
                              in_=_ap(w_hd.ap(), 0, [[2, 128], [256, km], [1, 2]]))
            bhd_sb = lw_p.tile([2, 1], FP32, tag="bhd")
            nc.sync.dma_start(out=bhd_sb[:], in_=_ap(b_hd.ap(), 0, [[1, 2], [1, 1]]))
            for si, (t0, ts_) in enumerate(spans):
                h_s = h_p.tile([128, km, tcmax], BF16, tag="h", name=f"hH_{si}")
                for k in range(km):
                    nc.sync.dma_start(out=h_s[:, k, :ts_],
                                      in_=hn_slices[si][k * 128:(k + 1) * 128, :ts_])
                ph = ps.tile([128, 512], FP32, tag="ps")
                for k in range(km):
                    nc.tensor.matmul(out=ph[0:2, :ts_], lhsT=whd_sb[:, k, :],
                                     rhs=h_s[:, k, :ts_],
                                     start=(k == 0), stop=(k == km - 1))
                yb = sm_p.tile([2, tcmax], FP32, tag="yb")
                nc.scalar.activation(out=yb[:, :ts_], in_=ph[0:2, :ts_],
                                     func=AF.Identity, bias=bhd_sb[:])
                lo = 1 if si == 0 else 0
                nc.sync.dma_start(out=yout[:, t0 + lo - 1:t0 + ts_ - 1],
                                  in_=yb[:, lo:ts_])

    nc.compile()
    return nc


def make_in_maps(cfg, inputs):
    """Host-side sharding: slice/transpose full inputs into per-core maps."""
    import ml_dtypes

    bf16 = ml_dtypes.bfloat16
    f32 = np.float32
    dh = cfg["d_half"]
    L = cfg["L"]
    n_cores = cfg["n_cores"]

    x_inputs = np.asarray(inputs["x_inputs"], f32)
    v_inputs = np.asarray(inputs["v_inputs"], f32)
    ipw = np.asarray(inputs["in_proj_w"], f32)
    d_inner = ipw.shape[1] // 2
    dm = np.asarray(inputs["x_emb_w"]).shape[0]
    in_maps = []
    for c in range(n_cores):
        b = c // 2
        h = c % 2
        sl = slice(h * dh, (h + 1) * dh)
        # xv6: rows [x0,x1, v0,v1, 1@t=0, 1@t>0]
        xv = np.zeros((6, L), f32)
        xv[0:2, 0] = x_inputs[b]
        xv[2:4, 1:] = v_inputs[b].T
        xv[4, 0] = 1.0
        xv[5, 1:] = 1.0
        # w6: rows [x_emb_w cols, v_proj_w cols, x_emb_b, v_proj_b] -> (6, dm)
        w6 = np.zeros((6, dm), f32)
        w6[0:2] = np.asarray(inputs["x_emb_w"], f32).T
        w6[2:4] = np.asarray(inputs["v_proj_w"], f32).T
        w6[4] = np.asarray(inputs["x_emb_b"], f32)
        w6[5] = np.asarray(inputs["v_proj_b"], f32)
        w_in_h = np.concatenate(
            [ipw[:, sl, :], ipw[:, d_inner + h * dh:d_inner + (h + 1) * dh, :]],
            axis=1)  # (nl, 2*dh, dm)
        m = {
            "xv6": xv.astype(bf16),
            "w6": w6.astype(bf16),
            "w_in": np.ascontiguousarray(w_in_h.transpose(0, 2, 1)).astype(bf16),
            "w_cv": np.ascontiguousarray(np.asarray(inputs["conv_w"], f32)[:, sl, 0, :]),
            "b_cv": np.ascontiguousarray(np.asarray(inputs["conv_b"], f32)[:, sl]),
            "w_xp": np.ascontiguousarray(
                np.asarray(inputs["x_proj_w"], f32)[:, :, sl].transpose(0, 2, 1)
            ).astype(bf16),
            "w_dt": np.ascontiguousarray(
                np.asarray(inputs["dt_proj_w"], f32)[:, sl, :].transpose(0, 2, 1)
            ).astype(bf16),
            "b_dt": np.ascontiguousarray(np.asarray(inputs["dt_proj_b"], f32)[:, sl]),
            "alog": np.ascontiguousarray(np.asarray(inputs["A_log"], f32)[:, sl, :]),
            "dskp": np.ascontiguousarray(np.asarray(inputs["D_skip"], f32)[:, sl]),
            "w_ou": np.ascontiguousarray(
                np.asarray(inputs["out_proj_w"], f32)[:, :, sl].transpose(0, 2, 1)
            ).astype(bf16),
            "w_hd": np.ascontiguousarray(np.asarray(inputs["head_w"], f32).T
                                         ).astype(bf16),
            "b_hd": np.ascontiguousarray(np.asarray(inputs["head_b"], f32)),
        }
        in_maps.append(m)
    return in_maps


_CACHE = {}


def _get_nc(cfg_key, cfg):
    if cfg_key not in _CACHE:
        _CACHE[cfg_key] = build(cfg)
    return _CACHE[cfg_key]


def run(inputs, trace=False, cfg=None, cfg_key="full"):
    from concourse.bass_utils import run_bass_kernel_spmd

    if cfg is None:
        cfg = full_cfg()
    nc = _get_nc(cfg_key, cfg)
    in_maps = make_in_maps(cfg, inputs)
    res = run_bass_kernel_spmd(nc, in_maps, core_ids=list(range(cfg["n_cores"])),
                               trace=trace)
    nb = cfg["n_cores"] // 2
    outs = [np.asarray(res.results[2 * b]["yout"], np.float32).T for b in range(nb)]
    return np.stack(outs, axis=0).astype(np.float32), res


def kernel(**inputs) -> np.ndarray:
    out, _ = run(inputs, trace=False)
    return out


# revision 7
# speedup vs baseline: 1.2740x; 1.2740x over previous
"""Trainium2 Bass kernel for nn_ContinuousMamba (v3).

Sharding: 8 cores = 4 batches x 2 halves of d_inner (1536 -> 768/core).
Core c handles batch c//2, channel half c%2; the pair AllReduces the
x_proj partial and the out_proj partial per time-slice (bf16).

v3 design:
- bf16 off the PE/psum path (bf16 matmul is 1-pass vs fp32 2-pass; DVE gets
  2x mode on packed bf16).
- The depthwise conv is folded into the in_proj x-matmuls: 4 time-shifted
  matmul accumulations with host-prescaled tap weights (diag(conv_w_k) @ W_x),
  reading h strips stored with a 3-column halo. u = silu(psum + conv_b).
- Each layer runs phase A (all 3 time strips: in_proj+conv, x_proj,
  AllReduce, delta) before phase C (scan strips), so the gpsimd/scalar queues
  never gate the next strip's collective or softplus.
- dA = exp(A*delta) via per-(m,n) scaled EXP on the scalar engine; the
  dBu broadcast-mult and the first level of the n-add-tree run on GpSimd;
  the scan, C*H mult and remaining tree levels on vector.
"""

import sys

sys.path.insert(0, "/opt/trn_rl_repo")

import numpy as np

import concourse.bass as bass
import concourse.tile as tile
from concourse import bacc, mybir
from concourse.bass import AP

FP32 = mybir.dt.float32
BF16 = mybir.dt.bfloat16
AF = mybir.ActivationFunctionType
OP = mybir.AluOpType


def full_cfg():
    return dict(
        n_cores=8,
        d_model=768,
        d_half=768,
        d_state=16,
        dt_rank=48,
        n_layers=4,
        L=1025,  # 1 x-token + 1024 v-tokens
        d_conv=4,
        strips=[342, 342, 341],
        dbu_on_gp=True,
        cmul_on_gp=False,
        tree1_on_gp=True,
    )


def _ap(base: AP, extra_offset: int, dims):
    return AP(tensor=base.tensor, offset=base.offset + extra_offset, ap=list(dims))


def build(cfg):
    dm = cfg["d_model"]
    dh = cfg["d_half"]
    ns = cfg["d_state"]
    dtr = cfg["dt_rank"]
    nl = cfg["n_layers"]
    L = cfg["L"]
    dconv = cfg["d_conv"]
    strips = cfg["strips"]
    assert sum(strips) == L
    km = dm // 128
    mh = dh // 128
    tcmax = max(strips)
    hw = dconv - 1  # halo width
    nx = dtr + 2 * ns  # 80
    nstrip = len(strips)
    spans = []
    t0 = 0
    for ts_ in strips:
        spans.append((t0, ts_))
        t0 += ts_

    nc = bacc.Bacc("TRN2", target_bir_lowering=False, debug=False,
                   num_devices=cfg["n_cores"])

    # ---- I/O ----
    xv6 = nc.dram_tensor("xv6", [6, L], BF16, kind="ExternalInput")
    w6 = nc.dram_tensor("w6", [6, dm], BF16, kind="ExternalInput")
    # conv-prescaled in_proj x weights: (nl, dconv, dm, dh)
    w_inx = nc.dram_tensor("w_inx", [nl, dconv, dm, dh], BF16, kind="ExternalInput")
    w_inz = nc.dram_tensor("w_inz", [nl, dm, dh], BF16, kind="ExternalInput")
    b_cv = nc.dram_tensor("b_cv", [nl, dh], FP32, kind="ExternalInput")
    w_xp = nc.dram_tensor("w_xp", [nl, dh, nx], BF16, kind="ExternalInput")
    w_dt = nc.dram_tensor("w_dt", [nl, dtr, dh], BF16, kind="ExternalInput")
    b_dt = nc.dram_tensor("b_dt", [nl, dh], FP32, kind="ExternalInput")
    alog = nc.dram_tensor("alog", [nl, dh, ns], FP32, kind="ExternalInput")
    dskp = nc.dram_tensor("dskp", [nl, dh], FP32, kind="ExternalInput")
    w_ou = nc.dram_tensor("w_ou", [nl, dh, dm], BF16, kind="ExternalInput")
    w_hd = nc.dram_tensor("w_hd", [dm, 2], BF16, kind="ExternalInput")
    b_hd = nc.dram_tensor("b_hd", [2], FP32, kind="ExternalInput")
    yout = nc.dram_tensor("yout", [2, L - 1], FP32, kind="ExternalOutput")

    with tile.TileContext(nc) as tc:
        import contextlib

        ctx = contextlib.ExitStack()
        with ctx:
            strip_p = ctx.enter_context(tc.tile_pool(name="strip", bufs=3))
            duy_p = ctx.enter_context(tc.tile_pool(name="duy", bufs=2))
            h_p = ctx.enter_context(tc.tile_pool(name="hp", bufs=4))
            scan_p = ctx.enter_context(tc.tile_pool(name="scan", bufs=2))
            hsc_p = ctx.enter_context(tc.tile_pool(name="hsc", bufs=1))
            bc_p = ctx.enter_context(tc.tile_pool(name="bc", bufs=1))
            w_p = ctx.enter_context(tc.tile_pool(name="w", bufs=2))
            lw_p = ctx.enter_context(tc.tile_pool(name="lw", bufs=2))
            sm_p = ctx.enter_context(tc.tile_pool(name="sm", bufs=2))
            carry_p = ctx.enter_context(tc.tile_pool(name="carry", bufs=2))
            ps = ctx.enter_context(tc.tile_pool(name="ps", bufs=6, space="PSUM"))
            dram = ctx.enter_context(tc.tile_pool(name="dram", bufs=4, space="DRAM"))

            groups = [[2 * i, 2 * i + 1] for i in range(cfg["n_cores"] // 2)]

            # ---------- embeddings -> h strips with halo (layer 0 input) ----
            xv_sb = lw_p.tile([6, L], BF16, tag="xv")
            nc.sync.dma_start(out=xv_sb[:], in_=xv6[:])
            w6_sb = lw_p.tile([6, dm], BF16, tag="w6")
            nc.sync.dma_start(out=w6_sb[:], in_=w6[:])

            h_tiles = [None] * nstrip
            for si, (t0, ts_) in enumerate(spans):
                h_s = h_p.tile([128, km, hw + tcmax], BF16, tag="h", name=f"h0_{si}")
                for k in range(km):
                    pt = ps.tile([128, 512], FP32, tag="ps")
                    nc.tensor.matmul(out=pt[:, :ts_],
                                     lhsT=w6_sb[:, k * 128:(k + 1) * 128],
                                     rhs=xv_sb[:, t0:t0 + ts_])
                    nc.scalar.copy(out=h_s[:, k, hw:hw + ts_], in_=pt[:, :ts_])
                    if si == 0:
                        nc.vector.memset(h_s[:, k, 0:hw], 0.0)
                    else:
                        nc.vector.tensor_copy(
                            out=h_s[:, k, 0:hw],
                            in_=h_tiles[si - 1][:, k, strips[si - 1]:strips[si - 1] + hw])
                h_tiles[si] = h_s

            # ---------- layers ----------
            hn_slices = [None] * nstrip
            for l in range(nl):
                with nc.named_scope(f"layer{l}"):
                    # --- per-layer small params ---
                    bcv_sb = lw_p.tile([128, mh], FP32, tag="bcv")
                    nc.sync.dma_start(out=bcv_sb[:],
                                      in_=_ap(b_cv.ap(), l * dh, [[1, 128], [128, mh]]))
                    bdt_sb = lw_p.tile([128, mh], FP32, tag="bdt")
                    nc.sync.dma_start(out=bdt_sb[:],
                                      in_=_ap(b_dt.ap(), l * dh, [[1, 128], [128, mh]]))
                    dsk_sb = lw_p.tile([128, mh], FP32, tag="dsk")
                    nc.sync.dma_start(out=dsk_sb[:],
                                      in_=_ap(dskp.ap(), l * dh, [[1, 128], [128, mh]]))
                    a_sb = lw_p.tile([128, mh, ns], FP32, tag="a")
                    nc.sync.dma_start(
                        out=a_sb[:],
                        in_=_ap(alog.ap(), l * dh * ns,
                                [[ns, 128], [128 * ns, mh], [1, ns]]))
                    a_flat = a_sb.rearrange("p m n -> p (m n)")
                    nc.scalar.activation(out=a_flat, in_=a_flat, func=AF.Exp)
                    nc.scalar.mul(out=a_flat, in_=a_flat, mul=-1.0)
                    wxp_sb = lw_p.tile([128, mh, nx], BF16, tag="wxp")
                    nc.sync.dma_start(
                        out=wxp_sb[:],
                        in_=_ap(w_xp.ap(), l * dh * nx,
                                [[nx, 128], [128 * nx, mh], [1, nx]]))
                    wdt_sb = lw_p.tile([dtr, dh], BF16, tag="wdt")
                    nc.sync.dma_start(out=wdt_sb[:],
                                      in_=_ap(w_dt.ap(), l * dtr * dh,
                                              [[dh, dtr], [1, dh]]))

                    # ---------------- phase A: all strips ----------------
                    u_t = [None] * nstrip
                    z_t = [None] * nstrip
                    dl_t = [None] * nstrip
                    du_t = [None] * nstrip
                    bball_t = [None] * nstrip
                    cball_t = [None] * nstrip
                    h_next = [None] * nstrip
                    for si, (t0, ts_) in enumerate(spans):
                        if l == 0:
                            h_s = h_tiles[si]
                        else:
                            h_s = h_p.tile([128, km, hw + tcmax], BF16, tag="h",
                                           name=f"h{l}_{si}")
                            for k in range(km):
                                nc.sync.dma_start(
                                    out=h_s[:, k, hw:hw + ts_],
                                    in_=hn_slices[si][k * 128:(k + 1) * 128, :ts_])
                                if si == 0:
                                    nc.vector.memset(h_s[:, k, 0:hw], 0.0)
                                else:
                                    nc.sync.dma_start(
                                        out=h_s[:, k, 0:hw],
                                        in_=hn_slices[si - 1][
                                            k * 128:(k + 1) * 128,
                                            strips[si - 1] - hw:strips[si - 1]])
                            h_next[si] = h_s
                        if l == 0:
                            h_next[si] = h_s

                        u_s = strip_p.tile([128, mh, tcmax], BF16, tag="u",
                                           name=f"u{l}_{si}")
                        z_s = strip_p.tile([128, mh, tcmax], BF16, tag="z",
                                           name=f"z{l}_{si}")
                        for m in range(mh):
                            winx = w_p.tile([128, dconv, km, 128], BF16, tag="winx")
                            nc.sync.dma_start(
                                out=winx[:],
                                in_=_ap(w_inx.ap(), l * dconv * dm * dh + m * 128,
                                        [[dh, 128], [dm * dh, dconv],
                                         [128 * dh, km], [1, 128]]))
                            winz = w_p.tile([128, km, 128], BF16, tag="winz")
                            nc.sync.dma_start(
                                out=winz[:],
                                in_=_ap(w_inz.ap(), l * dm * dh + m * 128,
                                        [[dh, 128], [128 * dh, km], [1, 128]]))
                            ptx = ps.tile([128, 512], FP32, tag="ps")
                            first = True
                            for c in range(dconv):
                                for k in range(km):
                                    nc.tensor.matmul(
                                        out=ptx[:, :ts_], lhsT=winx[:, c, k, :],
                                        rhs=h_s[:, k, c:c + ts_],
                                        start=first,
                                        stop=(c == dconv - 1 and k == km - 1))
                                    first = False
                            nc.scalar.activation(out=u_s[:, m, :ts_], in_=ptx[:, :ts_],
                                                 func=AF.Silu, bias=bcv_sb[:, m:m + 1])
                            ptz = ps.tile([128, 512], FP32, tag="ps")
                            for k in range(km):
                                nc.tensor.matmul(out=ptz[:, :ts_], lhsT=winz[:, k, :],
                                                 rhs=h_s[:, k, hw:hw + ts_],
                                                 start=(k == 0), stop=(k == km - 1))
                            nc.scalar.activation(out=z_s[:, m, :ts_], in_=ptz[:, :ts_],
                                                 func=AF.Silu)
                        u_t[si] = u_s
                        z_t[si] = z_s

                        # x_proj partial -> DRAM -> AllReduce
                        ptp = ps.tile([128, 512], FP32, tag="ps")
                        for k in range(mh):
                            nc.tensor.matmul(out=ptp[0:nx, :ts_], lhsT=wxp_sb[:, k, :],
                                             rhs=u_s[:, k, :ts_],
                                             start=(k == 0), stop=(k == mh - 1))
                        xpp_sb = sm_p.tile([nx, tcmax], BF16, tag="xpp")
                        nc.scalar.copy(out=xpp_sb[:, :ts_], in_=ptp[0:nx, :ts_])
                        xpp_d = dram.tile([nx, ts_], BF16, tag="xpp")
                        nc.sync.dma_start(out=xpp_d[:], in_=xpp_sb[:, :ts_])
                        xpr_d = dram.tile([nx, ts_], BF16, tag="xpr")
                        nc.gpsimd.collective_compute(
                            "AllReduce", OP.add, replica_groups=groups,
                            ins=[xpp_d.opt()], outs=[xpr_d.opt()])

                        # dt rows -> delta (softplus); B/C broadcast loads
                        xdt_sb = sm_p.tile([dtr, tcmax], BF16, tag="xdt")
                        nc.sync.dma_start(out=xdt_sb[:, :ts_], in_=xpr_d[0:dtr, :])
                        b_ball = bc_p.tile([128, ns * tcmax], BF16, tag="bball")
                        c_ball = bc_p.tile([128, ns * tcmax], BF16, tag="cball")
                        nc.sync.dma_start(
                            out=b_ball[:, :ns * ts_],
                            in_=_ap(xpr_d[:], dtr * ts_,
                                    [[0, 128], [ts_, ns], [1, ts_]]))
                        nc.sync.dma_start(
                            out=c_ball[:, :ns * ts_],
                            in_=_ap(xpr_d[:], (dtr + ns) * ts_,
                                    [[0, 128], [ts_, ns], [1, ts_]]))
                        bball_t[si] = b_ball
                        cball_t[si] = c_ball
                        dl_s = strip_p.tile([128, mh, tcmax], BF16, tag="dl",
                                            name=f"dl{l}_{si}")
                        for m in range(mh):
                            ptd = ps.tile([128, 512], FP32, tag="ps")
                            nc.tensor.matmul(out=ptd[:, :ts_],
                                             lhsT=wdt_sb[:, m * 128:(m + 1) * 128],
                                             rhs=xdt_sb[:, :ts_])
                            spe = sm_p.tile([128, tcmax], BF16, tag="spe")
                            nc.scalar.activation(out=spe[:, :ts_],
                                                 in_=ptd[:, :ts_], func=AF.Exp,
                                                 bias=bdt_sb[:, m:m + 1])
                            nc.scalar.activation(out=dl_s[:, m, :ts_],
                                                 in_=spe[:, :ts_], func=AF.Ln,
                                                 bias=1.0)
                        dl_t[si] = dl_s
                        du_s = duy_p.tile([128, mh, tcmax], BF16, tag="du",
                                          name=f"du{l}_{si}")
                        for m in range(mh):
                            nc.vector.tensor_mul(out=du_s[:, m, :ts_],
                                                 in0=dl_s[:, m, :ts_],
                                                 in1=u_s[:, m, :ts_])
                        du_t[si] = du_s

                    # ---------------- phase C: scan strips ----------------
                    carries = [None] * mh
                    for si, (t0, ts_) in enumerate(spans):
                        u_s, z_s, dl_s, du_s = u_t[si], z_t[si], dl_t[si], du_t[si]
                        b_ball, c_ball = bball_t[si], cball_t[si]
                        y_s = duy_p.tile([128, mh, tcmax], BF16, tag="y",
                                         name=f"y{l}_{si}")
                        for m in range(mh):
                            da = scan_p.tile([128, ns * tcmax], BF16, tag="da")
                            for n in range(ns):
                                nc.scalar.activation(
                                    out=da[:, n * ts_:(n + 1) * ts_],
                                    in_=dl_s[:, m, :ts_], func=AF.Exp,
                                    scale=a_sb[:, m, n:n + 1])
                            dbu = scan_p.tile([128, ns * tcmax], BF16, tag="dbu")
                            du_bc = _ap(du_s[:], m * tcmax,
                                        [du_s[:].ap[0], [0, ns], [1, ts_]])
                            eng = nc.gpsimd if cfg["dbu_on_gp"] else nc.vector
                            eng.tensor_tensor(out=dbu[:, :ns * ts_], in0=du_bc,
                                              in1=b_ball[:, :ns * ts_], op=OP.mult)
                            if si > 0:
                                da0 = sm_p.tile([128, ns], BF16, tag="da0")
                                nc.vector.tensor_copy(
                                    out=da0[:],
                                    in_=_ap(da[:], 0, [da[:].ap[0], [ts_, ns]]))
                                inj = sm_p.tile([128, ns], BF16, tag="inj")
                                nc.vector.tensor_mul(out=inj[:], in0=da0[:],
                                                     in1=carries[m][:])
                                dbu0 = _ap(dbu[:], 0, [dbu[:].ap[0], [ts_, ns]])
                                nc.vector.tensor_tensor(out=dbu0, in0=dbu0,
                                                        in1=inj[:], op=OP.add)
                            nc.vector.memset(
                                _ap(da[:], 0, [da[:].ap[0], [ts_, ns]]), 0.0)
                            hsc = hsc_p.tile([128, ns * tcmax], BF16, tag="hsc")
                            nc.vector.tensor_tensor_scan(
                                out=hsc[:, :ns * ts_], data0=da[:, :ns * ts_],
                                data1=dbu[:, :ns * ts_], initial=0.0,
                                op0=OP.mult, op1=OP.add)
                            if si < nstrip - 1:
                                ncar = carry_p.tile([128, ns], BF16, tag=f"car{m}",
                                                    name=f"car{l}_{si}_{m}")
                                nc.vector.tensor_copy(
                                    out=ncar[:],
                                    in_=_ap(hsc[:], ts_ - 1, [hsc[:].ap[0], [ts_, ns]]))
                                carries[m] = ncar
                            # ch = C * H (into dbu)
                            ceng = nc.gpsimd if cfg["cmul_on_gp"] else nc.vector
                            ceng.tensor_tensor(out=dbu[:, :ns * ts_],
                                               in0=hsc[:, :ns * ts_],
                                               in1=c_ball[:, :ns * ts_], op=OP.mult)
                            # add-tree over n: dbu -> hsc -> dbu -> hsc -> y
                            teng = nc.gpsimd if cfg["tree1_on_gp"] else nc.vector
                            teng.tensor_tensor(out=hsc[:, 0:8 * ts_],
                                               in0=dbu[:, 0:8 * ts_],
                                               in1=dbu[:, 8 * ts_:16 * ts_],
                                               op=OP.add)
                            nc.vector.tensor_add(out=dbu[:, 0:4 * ts_],
                                                 in0=hsc[:, 0:4 * ts_],
                                                 in1=hsc[:, 4 * ts_:8 * ts_])
                            nc.vector.tensor_add(out=hsc[:, 0:2 * ts_],
                                                 in0=dbu[:, 0:2 * ts_],
                                                 in1=dbu[:, 2 * ts_:4 * ts_])
                            nc.vector.tensor_add(out=y_s[:, m, :ts_],
                                                 in0=hsc[:, 0:ts_],
                                                 in1=hsc[:, ts_:2 * ts_])
                            # y += u * D_skip
                            uD = sm_p.tile([128, tcmax], BF16, tag="uD")
                            nc.vector.tensor_scalar_mul(out=uD[:, :ts_],
                                                        in0=u_s[:, m, :ts_],
                                                        scalar1=dsk_sb[:, m:m + 1])
                            nc.vector.tensor_add(out=y_s[:, m, :ts_],
                                                 in0=y_s[:, m, :ts_],
                                                 in1=uD[:, :ts_])
                        # y *= silu(z)
                        nc.vector.tensor_mul(
                            out=y_s.rearrange("p m t -> p (m t)"),
                            in0=y_s.rearrange("p m t -> p (m t)"),
                            in1=z_s.rearrange("p m t -> p (m t)"))

                        # out_proj partial for this strip -> AllReduce
                        hp_d = dram.tile([dm, ts_], BF16, tag="hpp")
                        for mo in range(km):
                            wou_t = w_p.tile([128, mh, 128], BF16, tag="wou")
                            nc.sync.dma_start(
                                out=wou_t[:],
                                in_=_ap(w_ou.ap(), l * dh * dm + mo * 128,
                                        [[dm, 128], [128 * dm, mh], [1, 128]]))
                            po = ps.tile([128, 512], FP32, tag="ps")
                            for k in range(mh):
                                nc.tensor.matmul(out=po[:, :ts_],
                                                 lhsT=wou_t[:, k, :],
                                                 rhs=y_s[:, k, :ts_],
                                                 start=(k == 0), stop=(k == mh - 1))
                            ho = sm_p.tile([128, tcmax], BF16, tag="ho")
                            nc.scalar.copy(out=ho[:, :ts_], in_=po[:, :ts_])
                            nc.sync.dma_start(out=hp_d[mo * 128:(mo + 1) * 128, :],
                                              in_=ho[:, :ts_])
                        hn_d = dram.tile([dm, ts_], BF16, tag="hnr")
                        nc.gpsimd.collective_compute(
                            "AllReduce", OP.add, replica_groups=groups,
                            ins=[hp_d.opt()], outs=[hn_d.opt()])
                        hn_slices[si] = hn_d

            # ---------- head ----------
            whd_sb = lw_p.tile([128, km, 2], BF16, tag="whd")
            nc.sync.dma_start(out=whd_sb[:],
                              in_=_ap(w_hd.ap(), 0, [[2, 128], [256, km], [1, 2]]))
            bhd_sb = lw_p.tile([2, 1], FP32, tag="bhd")
            nc.sync.dma_start(out=bhd_sb[:], in_=_ap(b_hd.ap(), 0, [[1, 2], [1, 1]]))
            for si, (t0, ts_) in enumerate(spans):
                h_s = h_p.tile([128, km, hw + tcmax], BF16, tag="h", name=f"hH_{si}")
                for k in range(km):
                    nc.sync.dma_start(out=h_s[:, k, hw:hw + ts_],
                                      in_=hn_slices[si][k * 128:(k + 1) * 128, :ts_])
                ph = ps.tile([128, 512], FP32, tag="ps")
                for k in range(km):
                    nc.tensor.matmul(out=ph[0:2, :ts_], lhsT=whd_sb[:, k, :],
                                     rhs=h_s[:, k, hw:hw + ts_],
                                     start=(k == 0), stop=(k == km - 1))
                yb = sm_p.tile([2, tcmax], FP32, tag="yb")
                nc.scalar.activation(out=yb[:, :ts_], in_=ph[0:2, :ts_],
                                     func=AF.Identity, bias=bhd_sb[:])
                lo = 1 if si == 0 else 0
                nc.sync.dma_start(out=yout[:, t0 + lo - 1:t0 + ts_ - 1],
                                  in_=yb[:, lo:ts_])

    nc.compile()
    return nc


def make_in_maps(cfg, inputs):
    """Host-side sharding: slice/transpose full inputs into per-core maps."""
    import ml_dtypes

    bf16 = ml_dtypes.bfloat16
    f32 = np.float32
    dh = cfg["d_half"]
    L = cfg["L"]
    dconv = cfg["d_conv"]
    n_cores = cfg["n_cores"]

    x_inputs = np.asarray(inputs["x_inputs"], f32)
    v_inputs = np.asarray(inputs["v_inputs"], f32)
    ipw = np.asarray(inputs["in_proj_w"], f32)
    cw = np.asarray(inputs["conv_w"], f32)  # (nl, d_inner, 1, dconv)
    d_inner = ipw.shape[1] // 2
    dm = np.asarray(inputs["x_emb_w"]).shape[0]
    in_maps = []
    for c in range(n_cores):
        b = c // 2
        h = c % 2
        sl = slice(h * dh, (h + 1) * dh)
        xv = np.zeros((6, L), f32)
        xv[0:2, 0] = x_inputs[b]
        xv[2:4, 1:] = v_inputs[b].T
        xv[4, 0] = 1.0
        xv[5, 1:] = 1.0
        w6 = np.zeros((6, dm), f32)
        w6[0:2] = np.asarray(inputs["x_emb_w"], f32).T
        w6[2:4] = np.asarray(inputs["v_proj_w"], f32).T
        w6[4] = np.asarray(inputs["x_emb_b"], f32)
        w6[5] = np.asarray(inputs["v_proj_b"], f32)
        # conv-prescaled x weights: (nl, dconv, dm, dh)
        wx = ipw[:, sl, :]  # (nl, dh, dm)
        wxc = np.einsum("ldm,ldc->lcmd", wx, cw[:, sl, 0, :])
        wz = ipw[:, d_inner + h * dh:d_inner + (h + 1) * dh, :]  # (nl, dh, dm)
        m = {
            "xv6": xv.astype(bf16),
            "w6": w6.astype(bf16),
            "w_inx": np.ascontiguousarray(wxc).astype(bf16),
            "w_inz": np.ascontiguousarray(wz.transpose(0, 2, 1)).astype(bf16),
            "b_cv": np.ascontiguousarray(np.asarray(inputs["conv_b"], f32)[:, sl]),
            "w_xp": np.ascontiguousarray(
                np.asarray(inputs["x_proj_w"], f32)[:, :, sl].transpose(0, 2, 1)
            ).astype(bf16),
            "w_dt": np.ascontiguousarray(
                np.asarray(inputs["dt_proj_w"], f32)[:, sl, :].transpose(0, 2, 1)
            ).astype(bf16),
            "b_dt": np.ascontiguousarray(np.asarray(inputs["dt_proj_b"], f32)[:, sl]),
            "alog": np.ascontiguousarray(np.asarray(inputs["A_log"], f32)[:, sl, :]),
            "dskp": np.ascontiguousarray(np.asarray(inputs["D_skip"], f32)[:, sl]),
            "w_ou": np.ascontiguousarray(
                np.asarray(inputs["out_proj_w"], f32)[:, :, sl].transpose(0, 2, 1)
            ).astype(bf16),
            "w_hd": np.ascontiguousarray(np.asarray(inputs["head_w"], f32).T
                                         ).astype(bf16),
            "b_hd": np.ascontiguousarray(np.asarray(inputs["head_b"], f32)),
        }
        in_maps.append(m)
    return in_maps


_CACHE = {}


def _get_nc(cfg_key, cfg):
    if cfg_key not in _CACHE:
        _CACHE[cfg_key] = build(cfg)
    return _CACHE[cfg_key]


def run(inputs, trace=False, cfg=None, cfg_key="full"):
    from concourse.bass_utils import run_bass_kernel_spmd

    if cfg is None:
        cfg = full_cfg()
    nc = _get_nc(cfg_key, cfg)
    in_maps = make_in_maps(cfg, inputs)
    res = run_bass_kernel_spmd(nc, in_maps, core_ids=list(range(cfg["n_cores"])),
                               trace=trace)
    nb = cfg["n_cores"] // 2
    outs = [np.asarray(res.results[2 * b]["yout"], np.float32).T for b in range(nb)]
    return np.stack(outs, axis=0).astype(np.float32), res


def kernel(**inputs) -> np.ndarray:
    out, _ = run(inputs, trace=False)
    return out


# revision 9
# speedup vs baseline: 1.9848x; 1.5579x over previous
"""Trainium2 Bass kernel for nn_ContinuousMamba (v4).

Sharding: 8 cores = 4 batches x 2 halves of d_inner (1536 -> 768/core).
Core c handles batch c//2, channel half c%2; the pair AllReduces the
x_proj partial and the out_proj partial per time-slice (bf16).

v4 design:
- bf16 off the PE/psum path (bf16 matmul is 1-pass vs fp32 2-pass; DVE gets
  2x mode on packed bf16).
- The depthwise conv is folded into the in_proj x-matmuls: 4 time-shifted
  matmul accumulations with host-prescaled tap weights (diag(conv_w_k) @ W_x),
  reading h strips stored with a 3-column halo. u = silu(psum + conv_b).
- ALL elementwise work runs on the vector engine: concurrent GpSimd tensor
  ops starve the DVE of SBUF bandwidth (measured 4.3x slowdown), so gpsimd
  only triggers collectives.
- Software-pipelined emission: layer l+1's phase A (in_proj/conv/x_proj/
  AllReduce/delta) for strip s is emitted right after layer l's phase C
  (scan/out_proj/AllReduce) of strip s, so collectives trigger early and
  tensor/scalar work overlaps the scan.
- dA = exp(A*delta) via per-(m,n) scaled EXP on the scalar engine; softplus
  via per-m EXP + one fused Ln over all m.
"""

import sys

sys.path.insert(0, "/opt/trn_rl_repo")

import numpy as np

import concourse.bass as bass
import concourse.tile as tile
from concourse import bacc, mybir
from concourse.bass import AP

FP32 = mybir.dt.float32
BF16 = mybir.dt.bfloat16
AF = mybir.ActivationFunctionType
OP = mybir.AluOpType


def full_cfg():
    return dict(
        n_cores=8,
        d_model=768,
        d_half=768,
        d_state=16,
        dt_rank=48,
        n_layers=4,
        L=1025,  # 1 x-token + 1024 v-tokens
        d_conv=4,
        strips=[342, 342, 341],
    )


def _ap(base: AP, extra_offset: int, dims):
    return AP(tensor=base.tensor, offset=base.offset + extra_offset, ap=list(dims))


def build(cfg):
    dm = cfg["d_model"]
    dh = cfg["d_half"]
    ns = cfg["d_state"]
    dtr = cfg["dt_rank"]
    nl = cfg["n_layers"]
    L = cfg["L"]
    dconv = cfg["d_conv"]
    strips = cfg["strips"]
    assert sum(strips) == L
    km = dm // 128
    mh = dh // 128
    tcmax = max(strips)
    hw = dconv - 1  # halo width
    nx = dtr + 2 * ns  # 80
    nstrip = len(strips)
    spans = []
    t0 = 0
    for ts_ in strips:
        spans.append((t0, ts_))
        t0 += ts_

    nc = bacc.Bacc("TRN2", target_bir_lowering=False, debug=False,
                   num_devices=cfg["n_cores"])

    # ---- I/O ----
    xv6 = nc.dram_tensor("xv6", [6, L], BF16, kind="ExternalInput")
    w6 = nc.dram_tensor("w6", [6, dm], BF16, kind="ExternalInput")
    # conv-prescaled in_proj x weights: (nl, dconv, dm, dh)
    w_inx = nc.dram_tensor("w_inx", [nl, dconv, dm, dh], BF16, kind="ExternalInput")
    w_inz = nc.dram_tensor("w_inz", [nl, dm, dh], BF16, kind="ExternalInput")
    b_cv = nc.dram_tensor("b_cv", [nl, dh], FP32, kind="ExternalInput")
    w_xp = nc.dram_tensor("w_xp", [nl, dh, nx], BF16, kind="ExternalInput")
    w_dt = nc.dram_tensor("w_dt", [nl, dtr, dh], BF16, kind="ExternalInput")
    b_dt = nc.dram_tensor("b_dt", [nl, dh], FP32, kind="ExternalInput")
    alog = nc.dram_tensor("alog", [nl, dh, ns], FP32, kind="ExternalInput")
    dskp = nc.dram_tensor("dskp", [nl, dh], FP32, kind="ExternalInput")
    w_ou = nc.dram_tensor("w_ou", [nl, dh, dm], BF16, kind="ExternalInput")
    w_hd = nc.dram_tensor("w_hd", [dm, 2], BF16, kind="ExternalInput")
    b_hd = nc.dram_tensor("b_hd", [2], FP32, kind="ExternalInput")
    yout = nc.dram_tensor("yout", [2, L - 1], FP32, kind="ExternalOutput")

    with tile.TileContext(nc) as tc:
        import contextlib

        ctx = contextlib.ExitStack()
        with ctx:
            strip_p = ctx.enter_context(tc.tile_pool(name="strip", bufs=3))
            duy_p = ctx.enter_context(tc.tile_pool(name="duy", bufs=3))
            h_p = ctx.enter_context(tc.tile_pool(name="hp", bufs=4))
            scan_p = ctx.enter_context(tc.tile_pool(name="scan", bufs=2))
            hsc_p = ctx.enter_context(tc.tile_pool(name="hsc", bufs=1))
            bc_p = ctx.enter_context(tc.tile_pool(name="bc", bufs=1))
            w_p = ctx.enter_context(tc.tile_pool(name="w", bufs=2))
            lw_p = ctx.enter_context(tc.tile_pool(name="lw", bufs=2))
            sm_p = ctx.enter_context(tc.tile_pool(name="sm", bufs=2))
            carry_p = ctx.enter_context(tc.tile_pool(name="carry", bufs=2))
            ps = ctx.enter_context(tc.tile_pool(name="ps", bufs=6, space="PSUM"))
            dram = ctx.enter_context(tc.tile_pool(name="dram", bufs=4, space="DRAM"))

            groups = [[2 * i, 2 * i + 1] for i in range(cfg["n_cores"] // 2)]

            hn_slices = [None] * nstrip  # DRAM h slices from previous layer
            params = {}  # per-layer small tensors
            A = {}  # phase-A products: (l, si) -> dict

            def load_params(l):
                p = {}
                p["bcv"] = lw_p.tile([128, mh], FP32, tag="bcv", name=f"bcv{l}")
                nc.sync.dma_start(out=p["bcv"][:],
                                  in_=_ap(b_cv.ap(), l * dh, [[1, 128], [128, mh]]))
                p["bdt"] = lw_p.tile([128, mh], FP32, tag="bdt", name=f"bdt{l}")
                nc.sync.dma_start(out=p["bdt"][:],
                                  in_=_ap(b_dt.ap(), l * dh, [[1, 128], [128, mh]]))
                p["dsk"] = lw_p.tile([128, mh], FP32, tag="dsk", name=f"dsk{l}")
                nc.sync.dma_start(out=p["dsk"][:],
                                  in_=_ap(dskp.ap(), l * dh, [[1, 128], [128, mh]]))
                a_sb = lw_p.tile([128, mh, ns], FP32, tag="a")
                nc.sync.dma_start(
                    out=a_sb[:],
                    in_=_ap(alog.ap(), l * dh * ns,
                            [[ns, 128], [128 * ns, mh], [1, ns]]))
                a_flat = a_sb.rearrange("p m n -> p (m n)")
                nc.scalar.activation(out=a_flat, in_=a_flat, func=AF.Exp)
                nc.scalar.mul(out=a_flat, in_=a_flat, mul=-1.0)
                p["a"] = a_sb
                p["wxp"] = lw_p.tile([128, mh, nx], BF16, tag="wxp", name=f"wxp{l}")
                nc.sync.dma_start(
                    out=p["wxp"][:],
                    in_=_ap(w_xp.ap(), l * dh * nx,
                            [[nx, 128], [128 * nx, mh], [1, nx]]))
                p["wdt"] = lw_p.tile([dtr, dh], BF16, tag="wdt", name=f"wdt{l}")
                nc.sync.dma_start(out=p["wdt"][:],
                                  in_=_ap(w_dt.ap(), l * dtr * dh,
                                          [[dh, dtr], [1, dh]]))
                params[l] = p

            def phase_a(l, si, h_s):
                """in_proj+conv -> u,z ; x_proj -> AllReduce ; delta, du, B/C."""
                t0, ts_ = spans[si]
                p = params[l]
                u_s = strip_p.tile([128, mh, tcmax], BF16, tag="u",
                                   name=f"u{l}_{si}")
                z_s = strip_p.tile([128, mh, tcmax], BF16, tag="z",
                                   name=f"z{l}_{si}")
                for m in range(mh):
                    winx = w_p.tile([128, dconv, km, 128], BF16, tag="winx")
                    nc.sync.dma_start(
                        out=winx[:],
                        in_=_ap(w_inx.ap(), l * dconv * dm * dh + m * 128,
                                [[dh, 128], [dm * dh, dconv],
                                 [128 * dh, km], [1, 128]]))
                    winz = w_p.tile([128, km, 128], BF16, tag="winz")
                    nc.sync.dma_start(
                        out=winz[:],
                        in_=_ap(w_inz.ap(), l * dm * dh + m * 128,
                                [[dh, 128], [128 * dh, km], [1, 128]]))
                    ptx = ps.tile([128, 512], FP32, tag="ps")
                    first = True
                    for c in range(dconv):
                        for k in range(km):
                            nc.tensor.matmul(
                                out=ptx[:, :ts_], lhsT=winx[:, c, k, :],
                                rhs=h_s[:, k, c:c + ts_],
                                start=first,
                                stop=(c == dconv - 1 and k == km - 1))
                            first = False
                    nc.scalar.activation(out=u_s[:, m, :ts_], in_=ptx[:, :ts_],
                                         func=AF.Silu, bias=p["bcv"][:, m:m + 1])
                    ptz = ps.tile([128, 512], FP32, tag="ps")
                    for k in range(km):
                        nc.tensor.matmul(out=ptz[:, :ts_], lhsT=winz[:, k, :],
                                         rhs=h_s[:, k, hw:hw + ts_],
                                         start=(k == 0), stop=(k == km - 1))
                    nc.scalar.activation(out=z_s[:, m, :ts_], in_=ptz[:, :ts_],
                                         func=AF.Silu)

                # x_proj partial -> DRAM -> AllReduce
                ptp = ps.tile([128, 512], FP32, tag="ps")
                for k in range(mh):
                    nc.tensor.matmul(out=ptp[0:nx, :ts_], lhsT=p["wxp"][:, k, :],
                                     rhs=u_s[:, k, :ts_],
                                     start=(k == 0), stop=(k == mh - 1))
                xpp_sb = sm_p.tile([nx, tcmax], BF16, tag="xpp")
                nc.scalar.copy(out=xpp_sb[:, :ts_], in_=ptp[0:nx, :ts_])
                xpp_d = dram.tile([nx, ts_], BF16, tag="xpp")
                nc.sync.dma_start(out=xpp_d[:], in_=xpp_sb[:, :ts_])
                xpr_d = dram.tile([nx, ts_], BF16, tag="xpr")
                nc.gpsimd.collective_compute(
                    "AllReduce", OP.add, replica_groups=groups,
                    ins=[xpp_d.opt()], outs=[xpr_d.opt()])

                # dt rows -> delta (softplus: per-m exp, one fused Ln)
                xdt_sb = sm_p.tile([dtr, tcmax], BF16, tag="xdt")
                nc.sync.dma_start(out=xdt_sb[:, :ts_], in_=xpr_d[0:dtr, :])
                b_ball = bc_p.tile([128, ns * tcmax], BF16, tag="bball")
                c_ball = bc_p.tile([128, ns * tcmax], BF16, tag="cball")
                nc.sync.dma_start(
                    out=b_ball[:, :ns * ts_],
                    in_=_ap(xpr_d[:], dtr * ts_, [[0, 128], [ts_, ns], [1, ts_]]))
                nc.sync.dma_start(
                    out=c_ball[:, :ns * ts_],
                    in_=_ap(xpr_d[:], (dtr + ns) * ts_,
                            [[0, 128], [ts_, ns], [1, ts_]]))
                spe6 = sm_p.tile([128, mh, tcmax], BF16, tag="spe6")
                for m in range(mh):
                    ptd = ps.tile([128, 512], FP32, tag="ps")
                    nc.tensor.matmul(out=ptd[:, :ts_],
                                     lhsT=p["wdt"][:, m * 128:(m + 1) * 128],
                                     rhs=xdt_sb[:, :ts_])
                    nc.scalar.activation(out=spe6[:, m, :ts_], in_=ptd[:, :ts_],
                                         func=AF.Exp, bias=p["bdt"][:, m:m + 1])
                dl_s = strip_p.tile([128, mh, tcmax], BF16, tag="dl",
                                    name=f"dl{l}_{si}")
                nc.scalar.activation(out=dl_s.rearrange("p m t -> p (m t)"),
                                     in_=spe6.rearrange("p m t -> p (m t)"),
                                     func=AF.Ln, bias=1.0)
                du_s = duy_p.tile([128, mh, tcmax], BF16, tag="du",
                                  name=f"du{l}_{si}")
                for m in range(mh):
                    nc.vector.tensor_mul(out=du_s[:, m, :ts_],
                                         in0=dl_s[:, m, :ts_],
                                         in1=u_s[:, m, :ts_])
                A[(l, si)] = dict(u=u_s, z=z_s, dl=dl_s, du=du_s,
                                  b=b_ball, c=c_ball)

            def load_h(l, si):
                """Load h strip (with halo) for layer l from hn_slices."""
                t0, ts_ = spans[si]
                h_s = h_p.tile([128, km, hw + tcmax], BF16, tag="h",
                               name=f"h{l}_{si}")
                for k in range(km):
                    nc.sync.dma_start(
                        out=h_s[:, k, hw:hw + ts_],
                        in_=hn_slices[si][k * 128:(k + 1) * 128, :ts_])
                    if si == 0:
                        nc.vector.memset(h_s[:, k, 0:hw], 0.0)
                    else:
                        nc.sync.dma_start(
                            out=h_s[:, k, 0:hw],
                            in_=hn_slices[si - 1][
                                k * 128:(k + 1) * 128,
                                strips[si - 1] - hw:strips[si - 1]])
                return h_s

            def phase_c(l, si, carries):
                """scan + gate + out_proj + AllReduce h for strip si."""
                t0, ts_ = spans[si]
                p = params[l]
                a = A.pop((l, si))
                u_s, z_s, dl_s, du_s = a["u"], a["z"], a["dl"], a["du"]
                b_ball, c_ball = a["b"], a["c"]
                y_s = duy_p.tile([128, mh, tcmax], BF16, tag="y", name=f"y{l}_{si}")
                for m in range(mh):
                    da = scan_p.tile([128, ns * tcmax], BF16, tag="da")
                    for n in range(ns):
                        nc.scalar.activation(
                            out=da[:, n * ts_:(n + 1) * ts_],
                            in_=dl_s[:, m, :ts_], func=AF.Exp,
                            scale=p["a"][:, m, n:n + 1])
                    dbu = scan_p.tile([128, ns * tcmax], BF16, tag="dbu")
                    du_bc = _ap(du_s[:], m * tcmax,
                                [du_s[:].ap[0], [0, ns], [1, ts_]])
                    nc.vector.tensor_tensor(out=dbu[:, :ns * ts_], in0=du_bc,
                                            in1=b_ball[:, :ns * ts_], op=OP.mult)
                    if si > 0:
                        da0 = sm_p.tile([128, ns], BF16, tag="da0")
                        nc.vector.tensor_copy(
                            out=da0[:], in_=_ap(da[:], 0, [da[:].ap[0], [ts_, ns]]))
                        inj = sm_p.tile([128, ns], BF16, tag="inj")
                        nc.vector.tensor_mul(out=inj[:], in0=da0[:],
                                             in1=carries[m][:])
                        dbu0 = _ap(dbu[:], 0, [dbu[:].ap[0], [ts_, ns]])
                        nc.vector.tensor_tensor(out=dbu0, in0=dbu0,
                                                in1=inj[:], op=OP.add)
                    nc.vector.memset(
                        _ap(da[:], 0, [da[:].ap[0], [ts_, ns]]), 0.0)
                    hsc = hsc_p.tile([128, ns * tcmax], BF16, tag="hsc")
                    nc.vector.tensor_tensor_scan(
                        out=hsc[:, :ns * ts_], data0=da[:, :ns * ts_],
                        data1=dbu[:, :ns * ts_], initial=0.0,
                        op0=OP.mult, op1=OP.add)
                    if si < nstrip - 1:
                        ncar = carry_p.tile([128, ns], BF16, tag=f"car{m}",
                                            name=f"car{l}_{si}_{m}")
                        nc.vector.tensor_copy(
                            out=ncar[:],
                            in_=_ap(hsc[:], ts_ - 1, [hsc[:].ap[0], [ts_, ns]]))
                        carries[m] = ncar
                    # ch = C * H (into dbu)
                    nc.vector.tensor_tensor(out=dbu[:, :ns * ts_],
                                            in0=hsc[:, :ns * ts_],
                                            in1=c_ball[:, :ns * ts_], op=OP.mult)
                    # y += u * D_skip: seed chain 0 of the tree with u*D
                    uD = sm_p.tile([128, tcmax], BF16, tag="uD")
                    nc.vector.tensor_scalar_mul(out=uD[:, :ts_],
                                                in0=u_s[:, m, :ts_],
                                                scalar1=p["dsk"][:, m:m + 1])
                    # add-tree over n: dbu -> hsc -> dbu -> hsc -> y
                    nc.vector.tensor_add(out=hsc[:, 0:8 * ts_],
                                         in0=dbu[:, 0:8 * ts_],
                                         in1=dbu[:, 8 * ts_:16 * ts_])
                    nc.vector.tensor_add(out=dbu[:, 0:4 * ts_],
                                         in0=hsc[:, 0:4 * ts_],
                                         in1=hsc[:, 4 * ts_:8 * ts_])
                    nc.vector.tensor_add(out=hsc[:, 0:2 * ts_],
                                         in0=dbu[:, 0:2 * ts_],
                                         in1=dbu[:, 2 * ts_:4 * ts_])
                    nc.vector.tensor_add(out=dbu[:, 0:ts_],
                                         in0=hsc[:, 0:ts_],
                                         in1=hsc[:, ts_:2 * ts_])
                    nc.vector.tensor_add(out=y_s[:, m, :ts_],
                                         in0=dbu[:, 0:ts_],
                                         in1=uD[:, :ts_])
                # y *= silu(z)
                nc.vector.tensor_mul(
                    out=y_s.rearrange("p m t -> p (m t)"),
                    in0=y_s.rearrange("p m t -> p (m t)"),
                    in1=z_s.rearrange("p m t -> p (m t)"))

                # out_proj partial for this strip -> AllReduce
                hp_d = dram.tile([dm, ts_], BF16, tag="hpp")
                for mo in range(km):
                    wou_t = w_p.tile([128, mh, 128], BF16, tag="wou")
                    nc.sync.dma_start(
                        out=wou_t[:],
                        in_=_ap(w_ou.ap(), l * dh * dm + mo * 128,
                                [[dm, 128], [128 * dm, mh], [1, 128]]))
                    po = ps.tile([128, 512], FP32, tag="ps")
                    for k in range(mh):
                        nc.tensor.matmul(out=po[:, :ts_], lhsT=wou_t[:, k, :],
                                         rhs=y_s[:, k, :ts_],
                                         start=(k == 0), stop=(k == mh - 1))
                    ho = sm_p.tile([128, tcmax], BF16, tag="ho")
                    nc.scalar.copy(out=ho[:, :ts_], in_=po[:, :ts_])
                    nc.sync.dma_start(out=hp_d[mo * 128:(mo + 1) * 128, :],
                                      in_=ho[:, :ts_])
                hn_d = dram.tile([dm, ts_], BF16, tag="hnr")
                nc.gpsimd.collective_compute(
                    "AllReduce", OP.add, replica_groups=groups,
                    ins=[hp_d.opt()], outs=[hn_d.opt()])
                hn_slices[si] = hn_d

            # ---------- embeddings -> h strips with halo (layer 0 input) ----
            xv_sb = lw_p.tile([6, L], BF16, tag="xv")
            nc.sync.dma_start(out=xv_sb[:], in_=xv6[:])
            w6_sb = lw_p.tile([6, dm], BF16, tag="w6")
            nc.sync.dma_start(out=w6_sb[:], in_=w6[:])

            h_tiles = [None] * nstrip
            for si, (t0, ts_) in enumerate(spans):
                h_s = h_p.tile([128, km, hw + tcmax], BF16, tag="h", name=f"h0_{si}")
                for k in range(km):
                    pt = ps.tile([128, 512], FP32, tag="ps")
                    nc.tensor.matmul(out=pt[:, :ts_],
                                     lhsT=w6_sb[:, k * 128:(k + 1) * 128],
                                     rhs=xv_sb[:, t0:t0 + ts_])
                    nc.scalar.copy(out=h_s[:, k, hw:hw + ts_], in_=pt[:, :ts_])
                    if si == 0:
                        nc.vector.memset(h_s[:, k, 0:hw], 0.0)
                    else:
                        nc.vector.tensor_copy(
                            out=h_s[:, k, 0:hw],
                            in_=h_tiles[si - 1][:, k,
                                                strips[si - 1]:strips[si - 1] + hw])
                h_tiles[si] = h_s

            # ---------- software-pipelined layers ----------
            load_params(0)
            for si in range(nstrip):
                phase_a(0, si, h_tiles[si])
            for l in range(nl):
                with nc.named_scope(f"layer{l}"):
                    if l + 1 < nl:
                        load_params(l + 1)
                    carries = [None] * mh
                    for si in range(nstrip):
                        phase_c(l, si, carries)
                        if l + 1 < nl:
                            phase_a(l + 1, si, load_h(l + 1, si))

            # ---------- head ----------
            whd_sb = lw_p.tile([128, km, 2], BF16, tag="whd")
            nc.sync.dma_start(out=whd_sb[:],
                              in_=_ap(w_hd.ap(), 0, [[2, 128], [256, km], [1, 2]]))
            bhd_sb = lw_p.tile([2, 1], FP32, tag="bhd")
            nc.sync.dma_start(out=bhd_sb[:], in_=_ap(b_hd.ap(), 0, [[1, 2], [1, 1]]))
            for si, (t0, ts_) in enumerate(spans):
                h_s = h_p.tile([128, km, hw + tcmax], BF16, tag="h", name=f"hH_{si}")
                for k in range(km):
                    nc.sync.dma_start(out=h_s[:, k, hw:hw + ts_],
                                      in_=hn_slices[si][k * 128:(k + 1) * 128, :ts_])
                ph = ps.tile([128, 512], FP32, tag="ps")
                for k in range(km):
                    nc.tensor.matmul(out=ph[0:2, :ts_], lhsT=whd_sb[:, k, :],
                                     rhs=h_s[:, k, hw:hw + ts_],
                                     start=(k == 0), stop=(k == km - 1))
                yb = sm_p.tile([2, tcmax], FP32, tag="yb")
                nc.scalar.activation(out=yb[:, :ts_], in_=ph[0:2, :ts_],
                                     func=AF.Identity, bias=bhd_sb[:])
                lo = 1 if si == 0 else 0
                nc.sync.dma_start(out=yout[:, t0 + lo - 1:t0 + ts_ - 1],
                                  in_=yb[:, lo:ts_])

    nc.compile()
    return nc


def make_in_maps(cfg, inputs):
    """Host-side sharding: slice/transpose full inputs into per-core maps."""
    import ml_dtypes

    bf16 = ml_dtypes.bfloat16
    f32 = np.float32
    dh = cfg["d_half"]
    L = cfg["L"]
    n_cores = cfg["n_cores"]

    x_inputs = np.asarray(inputs["x_inputs"], f32)
    v_inputs = np.asarray(inputs["v_inputs"], f32)
    ipw = np.asarray(inputs["in_proj_w"], f32)
    cw = np.asarray(inputs["conv_w"], f32)  # (nl, d_inner, 1, dconv)
    d_inner = ipw.shape[1] // 2
    dm = np.asarray(inputs["x_emb_w"]).shape[0]
    in_maps = []
    for c in range(n_cores):
        b = c // 2
        h = c % 2
        sl = slice(h * dh, (h + 1) * dh)
        xv = np.zeros((6, L), f32)
        xv[0:2, 0] = x_inputs[b]
        xv[2:4, 1:] = v_inputs[b].T
        xv[4, 0] = 1.0
        xv[5, 1:] = 1.0
        w6 = np.zeros((6, dm), f32)
        w6[0:2] = np.asarray(inputs["x_emb_w"], f32).T
        w6[2:4] = np.asarray(inputs["v_proj_w"], f32).T
        w6[4] = np.asarray(inputs["x_emb_b"], f32)
        w6[5] = np.asarray(inputs["v_proj_b"], f32)
        wx = ipw[:, sl, :]  # (nl, dh, dm)
        wxc = np.einsum("ldm,ldc->lcmd", wx, cw[:, sl, 0, :])
        wz = ipw[:, d_inner + h * dh:d_inner + (h + 1) * dh, :]
        m = {
            "xv6": xv.astype(bf16),
            "w6": w6.astype(bf16),
            "w_inx": np.ascontiguousarray(wxc).astype(bf16),
            "w_inz": np.ascontiguousarray(wz.transpose(0, 2, 1)).astype(bf16),
            "b_cv": np.ascontiguousarray(np.asarray(inputs["conv_b"], f32)[:, sl]),
            "w_xp": np.ascontiguousarray(
                np.asarray(inputs["x_proj_w"], f32)[:, :, sl].transpose(0, 2, 1)
            ).astype(bf16),
            "w_dt": np.ascontiguousarray(
                np.asarray(inputs["dt_proj_w"], f32)[:, sl, :].transpose(0, 2, 1)
            ).astype(bf16),
            "b_dt": np.ascontiguousarray(np.asarray(inputs["dt_proj_b"], f32)[:, sl]),
            "alog": np.ascontiguousarray(np.asarray(inputs["A_log"], f32)[:, sl, :]),
            "dskp": np.ascontiguousarray(np.asarray(inputs["D_skip"], f32)[:, sl]),
            "w_ou": np.ascontiguousarray(
                np.asarray(inputs["out_proj_w"], f32)[:, :, sl].transpose(0, 2, 1)
            ).astype(bf16),
            "w_hd": np.ascontiguousarray(np.asarray(inputs["head_w"], f32).T
                                         ).astype(bf16),
            "b_hd": np.ascontiguousarray(np.asarray(inputs["head_b"], f32)),
        }
        in_maps.append(m)
    return in_maps


_CACHE = {}


def _get_nc(cfg_key, cfg):
    if cfg_key not in _CACHE:
        _CACHE[cfg_key] = build(cfg)
    return _CACHE[cfg_key]


def run(inputs, trace=False, cfg=None, cfg_key="full"):
    from concourse.bass_utils import run_bass_kernel_spmd

    if cfg is None:
        cfg = full_cfg()
    nc = _get_nc(cfg_key, cfg)
    in_maps = make_in_maps(cfg, inputs)
    res = run_bass_kernel_spmd(nc, in_maps, core_ids=list(range(cfg["n_cores"])),
                               trace=trace)
    nb = cfg["n_cores"] // 2
    outs = [np.asarray(res.results[2 * b]["yout"], np.float32).T for b in range(nb)]
    return np.stack(outs, axis=0).astype(np.float32), res


def kernel(**inputs) -> np.ndarray:
    out, _ = run(inputs, trace=False)
    return out
